# revision 17
# baseline (speedup 1.0000x reference)
"""MinGRU layer kernel for Trainium2 (8 NeuronCores, data-parallel over batch).

Math per batch element b (reference semantics):
    z_t = Wz @ x_t + bz ; g_t = sigmoid(z_t)
    u_t = Wh @ x_t + bh
    h_t = (1-g_t) * h_{t-1} + g_t * u_t     (linear recurrence along T)
    y_t = Wo @ h_t + bo
Device layout: hidden dim on partitions (8 tiles x 128), time on the free
dim, chunked by TC=512 columns. The recurrence runs on the DVE
``tensor_tensor_scan`` instruction (state = a*state + b along the free dim)
with a = sigmoid(-z-bz) = 1-g and b = (u+bh)*g.

Precision plan (validated against the reference in fp-exact simulation,
rel_l2 = 1.78e-2 < 2e-2): the z matmul runs entirely in fp8 DoubleRow
(2x PE throughput; the sigmoid's g*(1-g) <= 0.25 slope attenuates the fp8
noise), u and o matmuls in bf16 with fp32 PSUM accumulation (fp8 there
costs ~3.8e-2 rel err - over budget). h is stored bf16; out is stored
bf16 (host converts to f32).

Schedule: chunk 0 computes all eight z m-groups first (they need only the
small fp8 operands, ~1.6 MB) while the bf16 x / wh / wo stream in, then
the u groups; later chunks run z groups in quads then their u groups. The
PE pays ~195ns to enter a DoubleRow chain from bf16 (DR->DR group starts
are free), so batching z groups minimizes transitions; four per batch is
the deepest the 5-slot zu PSUM ring and the sigmoid drain rate allow
without micro-stalls (which collapse the PE clock ramp). Output-chunk
matmuls are deferred one chunk so the PE never waits on the serial scan
chain; the final chunk's stores spread over three DMA queues. g lives in
a single [P, MH, TC] buffer reused across chunks (the next chunk's
sigmoid naturally waits for the previous chunk's stt to read its slice).

Sharding: batch B=8 -> one batch element per core; weights broadcast.
"""

import numpy as np
import ml_dtypes

import concourse.bass as bass
import concourse.bacc as bacc
import concourse.mybir as mybir
import concourse.tile as tile
from concourse.bass_utils import run_bass_kernel_spmd
from concourse.bass_interp import get_hw_module
from concourse.tile_rust import add_dep_helper

B, T, I, H, O = 8, 4096, 1024, 1024, 1024
P = 128
TC = 512  # time chunk (matmul free dim / PSUM bank)
SW = 64.0  # fp8 weight scale; the sigmoid ACT divides it out

BF16 = mybir.dt.bfloat16
F32 = mybir.dt.float32
FP8 = mybir.dt.float8e4
NPBF16 = ml_dtypes.bfloat16
NPFP8 = ml_dtypes.float8_e4m3fn

AL = mybir.AluOpType
AF = mybir.ActivationFunctionType
DR = mybir.MatmulPerfMode.DoubleRow


def build_program(t=T, i=I, h=H, o=O, tc_len=TC, n_cores=8, enable_asserts=False):
    KI, MH, MO, NCH = i // P, h // P, o // P, t // tc_len
    KP = i // (2 * P)  # fp8 DoubleRow k-pair count for the z matmul
    nc = bacc.Bacc(
        "TRN2",
        target_bir_lowering=False,
        debug=False,
        enable_asserts=enable_asserts,
        num_devices=n_cores,
    )

    # Host pre-tiled layouts (see kernel() below for the exact packing).
    xT = nc.dram_tensor("xT", [P, NCH, KI, tc_len], BF16, kind="ExternalInput")
    # fp8 moving operand for the z DoubleRow matmuls: planar k-subrow
    # pairs ([p, c, kp, j, t] = x[(2*kp+j)*P+p, c*TC+t]).
    x8d = nc.dram_tensor("x8", [P, NCH, KP, 2, tc_len], FP8, kind="ExternalInput")
    wz8d = nc.dram_tensor("wz8", [P, MH, KP, 2, P], FP8, kind="ExternalInput")
    wh = nc.dram_tensor("wh", [P, MH, KI, P], BF16, kind="ExternalInput")
    wo = nc.dram_tensor("wo", [P, MO, MH, P], BF16, kind="ExternalInput")
    # bz | nbz | bh | bo side by side so one DMA moves all biases.
    biasd = nc.dram_tensor("biases", [P, 3 * MH + MO], F32, kind="ExternalInput")
    out = nc.dram_tensor("out", [P, MO, t], BF16, kind="ExternalOutput")

    with tile.TileContext(nc, pool_alloc_mode="queue") as tcx:
        with (
            tcx.tile_pool(name="weights", bufs=1) as wpool,
            tcx.tile_pool(name="xin", bufs=2) as xpool,
            tcx.tile_pool(name="gtmp", bufs=4) as gpool,
            tcx.tile_pool(name="g0buf", bufs=1) as g0pool,
            tcx.tile_pool(name="ab", bufs=2) as abpool,
            tcx.tile_pool(name="hsb", bufs=3) as hpool,
            tcx.tile_pool(name="osb", bufs=8) as opool,
            tcx.tile_pool(name="zups", bufs=5, space=bass.MemorySpace.PSUM) as zups,
            tcx.tile_pool(name="ops", bufs=3, space=bass.MemorySpace.PSUM) as ops,
        ):
            x_first = xpool.tile([P, KI, tc_len], BF16, tag="x")
            x8_first = xpool.tile([P, KP, 2, tc_len], FP8, tag="x8")
            wz8_s = wpool.tile([P, MH, KP, 2, P], FP8, tag="wz8")
            wh_s = wpool.tile([P, MH, KI, P], BF16, tag="wh")
            wo_s = wpool.tile([P, MO, MH, P], BF16, tag="wo")
            bias_s = wpool.tile([P, 3 * MH + MO], F32, tag="biases")
            bz_s = bias_s[:, 0:MH]
            nbz_s = bias_s[:, MH : 2 * MH]
            bh_s = bias_s[:, 2 * MH : 3 * MH]
            bo_s = bias_s[:, 3 * MH : 3 * MH + MO]

            # Pre-warm the PE while the startup DMAs are in flight: the HAM
            # clock gate needs ~3us of continuous matmul work to reach full
            # speed. The scratch memset runs on the vector engine; the PSUM
            # tile comes from the (idle until ~37us) o ring, never read.
            warm_sb = gpool.tile([P, tc_len], BF16, tag="warm")
            nc.vector.memset(warm_sb[:], 0.0)
            warm_ps = ops.tile([P, tc_len], F32, tag="o")

            def emit_warm(n):
                for _ in range(n):
                    nc.tensor.matmul(
                        warm_ps[:], warm_sb[:, 0:P], warm_sb[:], start=True, stop=True
                    )

            emit_warm(8)

            # Critical startup transfers across three queues, each ordered by
            # consumption. The z phase of chunk 0 needs only x8 + wz8
            # (fp8, ~1.5 MB); x bf16 / wh stream in behind it for the u
            # phase; wo and later chunks are gated on compute progress. The
            # first ~25us are DMA-ramp-bound, so the early window carries
            # only bytes needed before ~25us (~6 MB).
            half = KI // 2
            nc.sync.dma_start(bias_s[:], biasd[:])
            nc.sync.dma_start(wz8_s[:, 0], wz8d[:, 0])
            nc.sync.dma_start(wz8_s[:, 1], wz8d[:, 1])
            for kp in range(KP):
                nc.scalar.dma_start(x8_first[:, kp], x8d[:, 0, kp])
            nc.gpsimd.dma_start(wh_s[:, 0], wh[:, 0])
            nc.gpsimd.dma_start(wh_s[:, 1], wh[:, 1])
            for kk in range(0, half, 2):
                nc.gpsimd.dma_start(x_first[:, kk : kk + 2], xT[:, 0, kk : kk + 2])
            nc.gpsimd.dma_start(wh_s[:, 2], wh[:, 2])
            nc.gpsimd.dma_start(wh_s[:, 3], wh[:, 3])

            def emit_out_chunk(c, h_tile, final=False):
                sl = slice(c * tc_len, (c + 1) * tc_len)
                for mo in range(MO):
                    if final and mo == MO - 1:
                        # The very last output tile is the kernel's critical
                        # tail. Accumulate it as two half-width PSUM groups
                        # with independent consumer chains on separate
                        # engines and DMA queues.
                        hl = tc_len // 2
                        sl_a = slice(c * tc_len, c * tc_len + hl)
                        sl_b = slice(c * tc_len + hl, (c + 1) * tc_len)
                        o_psa = ops.tile([P, tc_len], F32, tag="o")
                        o_psb = ops.tile([P, tc_len], F32, tag="o")
                        for k in range(MH):
                            nc.tensor.matmul(
                                o_psa[:, 0:hl],
                                wo_s[:, mo, k, :],
                                h_tile[:, k, 0:hl],
                                start=(k == 0),
                                stop=(k == MH - 1),
                            )
                            nc.tensor.matmul(
                                o_psb[:, 0:hl],
                                wo_s[:, mo, k, :],
                                h_tile[:, k, hl:tc_len],
                                start=(k == 0),
                                stop=(k == MH - 1),
                            )
                        o_sb = opool.tile([P, tc_len], BF16, tag="osb")
                        nc.vector.tensor_scalar_add(
                            o_sb[:, hl:tc_len], o_psb[:, 0:hl],
                            bo_s[:, mo : mo + 1],
                        )
                        nc.scalar.activation(
                            o_sb[:, 0:hl],
                            o_psa[:, 0:hl],
                            AF.Identity,
                            bias=bo_s[:, mo : mo + 1],
                        )
                        nc.scalar.dma_start(out[:, mo, sl_b], o_sb[:, hl:tc_len])
                        nc.sync.dma_start(out[:, mo, sl_a], o_sb[:, 0:hl])
                        continue
                    o_ps = ops.tile([P, tc_len], F32, tag="o")
                    for k in range(MH):
                        nc.tensor.matmul(
                            o_ps[:],
                            wo_s[:, mo, k, :],
                            h_tile[:, k, :],
                            start=(k == 0),
                            stop=(k == MH - 1),
                        )
                    o_sb = opool.tile([P, tc_len], BF16, tag="osb")
                    # Bias-add on the scalar engine: keeps o-PSUM recycling
                    # off the DVE queue (which carries the scan chain).
                    nc.scalar.activation(
                        o_sb[:], o_ps[:], AF.Identity, bias=bo_s[:, mo : mo + 1]
                    )
                    # Spread the final chunk's stores over three queues so
                    # the end-of-kernel DMA drain parallelizes.
                    q = (nc.sync, nc.scalar, nc.gpsimd)[mo % 3] if final else nc.sync
                    q.dma_start(out[:, mo, sl], o_sb[:])

            def emit_z(m, z_ps, x8_s, c):
                for kp in range(KP):
                    mm = nc.tensor.matmul(
                        z_ps[:],
                        wz8_s[:, m, kp],
                        x8_s[:, kp],
                        start=(kp == 0),
                        stop=(kp == KP - 1),
                        perf_mode=DR,
                    )
                    if c == 0 and m == 0 and kp == 0 and MH > 2:
                        d = nc.sync.dma_start(wz8_s[:, 2:MH], wz8d[:, 2:MH])
                        add_dep_helper(d.ins, mm.ins, True, "wz8 bulk after start")
                return mm

            def emit_u(m, u_ps, x_s):
                for k in range(KI):
                    mm = nc.tensor.matmul(
                        u_ps[:],
                        wh_s[:, m, k, :],
                        x_s[:, k, :],
                        start=(k == 0),
                        stop=(k == KI - 1),
                    )
                return mm

            def emit_gates(m, z_ps, g_dst, a_s):
                # g = sigmoid(z + bz); a = 1 - g = sigmoid(-z - bz)
                nc.scalar.activation(
                    g_dst, z_ps[:], AF.Sigmoid, bias=bz_s[:, m : m + 1],
                    scale=1.0 / SW,
                )
                nc.scalar.activation(
                    a_s[:, m, :], z_ps[:], AF.Sigmoid, bias=nbz_s[:, m : m + 1],
                    scale=-1.0 / SW,
                )

            def emit_scan(m, c, u_ps, g_src, a_s, b_s, h_s, h_prev):
                # b = (u + bh) * g
                nc.vector.scalar_tensor_tensor(
                    b_s[:, m, :], u_ps[:], bh_s[:, m : m + 1], g_src, AL.add, AL.mult
                )
                # h[:, t] = a[:, t] * h[:, t-1] + b[:, t]
                init = 0.0 if c == 0 else h_prev[:, m, tc_len - 1 : tc_len]
                nc.vector.tensor_tensor_scan(
                    h_s[:, m, :], a_s[:, m, :], b_s[:, m, :], init, AL.mult, AL.add
                )

            # ---- chunk 0: z-first phase (fp8-only operands), then u phase.
            a_s = abpool.tile([P, MH, tc_len], F32, tag="a")
            b_s = abpool.tile([P, MH, tc_len], F32, tag="b")
            h_s = hpool.tile([P, MH, tc_len], BF16, tag="h")
            g0 = g0pool.tile([P, MH, tc_len], F32, tag="g0")
            x8_n = None
            for m in range(MH):
                z_ps = zups.tile([P, tc_len], F32, tag="zu")
                mm = emit_z(m, z_ps, x8_first, 0)
                if m == 1:
                    # back half of x chunk 0 behind the x8 loads, and the
                    # remaining u weights.
                    for kk in range(half, KI, 2):
                        d = nc.scalar.dma_start(
                            x_first[:, kk : kk + 2], xT[:, 0, kk : kk + 2]
                        )
                        add_dep_helper(d.ins, mm.ins, True, "x c0 back half")
                    for mw in range(4, MH):
                        d = nc.gpsimd.dma_start(wh_s[:, mw], wh[:, mw])
                        add_dep_helper(d.ins, mm.ins, True, "wh bulk")
                if m == MH - 1 and NCH > 1:
                    # next chunk's fp8 x on the (now idle) sync queue so
                    # chunk 1's z groups aren't starved.
                    x8_n = xpool.tile([P, KP, 2, tc_len], FP8, tag="x8")
                    d = nc.sync.dma_start(x8_n[:], x8d[:, 1])
                    add_dep_helper(d.ins, mm.ins, True, "x8 c1 after z phase")
                emit_gates(m, z_ps, g0[:, m, :], a_s)
            emit_warm(2)
            x_next = None
            for m in range(MH):
                u_ps = zups.tile([P, tc_len], F32, tag="zu")
                mm = emit_u(m, u_ps, x_first)
                if m == 0 and NCH > 1:
                    xb_n = xpool.tile([P, KI, tc_len], BF16, tag="x")
                    d = nc.gpsimd.dma_start(xb_n[:], xT[:, 1])
                    add_dep_helper(d.ins, mm.ins, True, "x c1 after u start")
                    x_next = (xb_n, x8_n)
                if m == 2:
                    d = nc.sync.dma_start(wo_s[:], wo[:])
                    add_dep_helper(d.ins, mm.ins, True, "wo after u start")
                emit_scan(m, 0, u_ps, g0[:, m, :], a_s, b_s, h_s, None)
            h_prev = h_s

            # ---- chunks 1..NCH-1: interleaved z,u per m-group.
            for c in range(1, NCH):
                if x_next is not None:
                    (x_s, x8_s), x_next = x_next, None
                else:
                    x8_s = xpool.tile([P, KP, 2, tc_len], FP8, tag="x8")
                    nc.gpsimd.dma_start(x8_s[:], x8d[:, c])
                    x_s = xpool.tile([P, KI, tc_len], BF16, tag="x")
                    nc.gpsimd.dma_start(x_s[:], xT[:, c])

                a_s = abpool.tile([P, MH, tc_len], F32, tag="a")
                b_s = abpool.tile([P, MH, tc_len], F32, tag="b")
                h_s = hpool.tile([P, MH, tc_len], BF16, tag="h")

                # z groups in quads: the PE pays ~195ns to enter a DR chain
                # from bf16 (DR->DR group starts are free), so batch four z
                # groups per transition. Four is the deepest batch whose
                # PSUM-ring / sigmoid-drain slacks stay ~1us (a full batch
                # of eight collapses the PE clock ramp on micro-stalls).
                for mp in range(0, MH, 4):
                    grp = range(mp, mp + 4)
                    zps_grp = []
                    for m in grp:
                        z_ps = zups.tile([P, tc_len], F32, tag="zu")
                        emit_z(m, z_ps, x8_s, c)
                        zps_grp.append(z_ps)
                        emit_gates(m, z_ps, g0[:, m, :], a_s)
                    for m in grp:
                        u_ps = zups.tile([P, tc_len], F32, tag="zu")
                        emit_u(m, u_ps, x_s)
                        emit_scan(m, c, u_ps, g0[:, m, :], a_s, b_s, h_s, h_prev)

                # Output matmuls for the previous chunk, emitted after this
                # chunk's gate/update matmuls so the PE stream never has to
                # wait on the (serial) scan chain.
                emit_out_chunk(c - 1, h_prev)
                h_prev = h_s
            emit_out_chunk(NCH - 1, h_prev, final=True)

    nc.compile()
    return nc


_CACHED_NC = None


def _get_nc():
    global _CACHED_NC
    if _CACHED_NC is None:
        _CACHED_NC = build_program()
    return _CACHED_NC


# Set by test harnesses that want a profile: kernel() stores the raw
# BassKernelResults of the last run here when TRACE is truthy.
TRACE = False
LAST_RESULTS = None


def _pack_weight(w):
    # [out_dim, in_dim] -> lhsT tiles [P, M_tiles, K_tiles, P] where
    # arr[p, m, k, q] = w[m*P + q, k*P + p]
    kd, md = w.shape[1] // P, w.shape[0] // P
    return np.ascontiguousarray(
        w.T.reshape(kd, P, md, P).transpose(1, 2, 0, 3).astype(NPBF16)
    )


def kernel(**inputs):
    global LAST_RESULTS
    xs = np.asarray(inputs["xs"], np.float32)
    Wz = np.asarray(inputs["Wz"], np.float32)
    bz = np.asarray(inputs["bz"], np.float32)
    Wh = np.asarray(inputs["Wh"], np.float32)
    bh = np.asarray(inputs["bh"], np.float32)
    Wo = np.asarray(inputs["Wo"], np.float32)
    bo = np.asarray(inputs["bo"], np.float32)

    KI, MH, MO, NCH = I // P, H // P, O // P, T // TC
    KP = I // (2 * P)

    # z weights as fp8 DoubleRow tiles, scaled by SW (the sigmoid ACT
    # divides it back out).
    wz8_t = np.ascontiguousarray(
        (Wz * SW).reshape(MH, P, KP, 2, P).transpose(4, 0, 2, 3, 1).astype(NPFP8)
    )
    wh_t = _pack_weight(Wh)
    wo_t = _pack_weight(Wo)
    bias_p = np.concatenate(
        [
            bz.reshape(MH, P).T,
            (-bz).reshape(MH, P).T,
            bh.reshape(MH, P).T,
            bo.reshape(MO, P).T,
        ],
        axis=1,
    )
    bias_p = np.ascontiguousarray(bias_p, np.float32)

    in_maps = []
    for b in range(B):
        # [T, I] -> [P, NCH, KI, TC] with x[p, c, k, t] = xs[b, c*TC+t, k*P+p]
        xb = xs[b].astype(NPBF16).reshape(NCH, TC, KI, P)
        xb = np.ascontiguousarray(xb.transpose(3, 0, 2, 1))
        # fp8 planar k-pair moving operand: [p, c, kp, j, t]
        x8b = np.ascontiguousarray(
            xs[b].reshape(NCH, TC, KP, 2, P).transpose(4, 0, 2, 3, 1).astype(NPFP8)
        )
        in_maps.append(
            {
                "xT": xb,
                "x8": x8b,
                "wz8": wz8_t,
                "wh": wh_t,
                "wo": wo_t,
                "biases": bias_p,
            }
        )

    nc = _get_nc()
    old_m = nc.m
    nc.m = get_hw_module(nc.m)
    try:
        res = run_bass_kernel_spmd(
            nc, in_maps, core_ids=list(range(B)), trace=bool(TRACE)
        )
    finally:
        nc.m = old_m
    LAST_RESULTS = res

    out_full = np.empty((B, T, O), np.float32)
    for b in range(B):
        # [P, MO, T] bf16 -> [O, T] -> [T, O] f32
        ob = np.asarray(res.results[b]["out"], dtype=np.float32)
        out_full[b] = ob.transpose(1, 0, 2).reshape(O, T).T
    return out_full


# revision 18
# speedup vs baseline: 1.0095x; 1.0095x over previous
"""MinGRU layer kernel for Trainium2 (8 NeuronCores, data-parallel over batch).

Math per batch element b (reference semantics):
    z_t = Wz @ x_t + bz ; g_t = sigmoid(z_t)
    u_t = Wh @ x_t + bh
    h_t = (1-g_t) * h_{t-1} + g_t * u_t     (linear recurrence along T)
    y_t = Wo @ h_t + bo
Device layout: hidden dim on partitions (8 tiles x 128), time on the free
dim, chunked by TC=512 columns. The recurrence runs on the DVE
``tensor_tensor_scan`` instruction (state = a*state + b along the free dim)
with a = sigmoid(-z-bz) = 1-g and b = (u+bh)*g.

Precision plan (validated against the reference in fp-exact simulation,
rel_l2 = 1.78e-2 < 2e-2): the z matmul runs entirely in fp8 DoubleRow
(2x PE throughput; the sigmoid's g*(1-g) <= 0.25 slope attenuates the fp8
noise), u and o matmuls in bf16 with fp32 PSUM accumulation (fp8 there
costs ~3.8e-2 rel err - over budget). h is stored bf16; out is stored
bf16 (host converts to f32).

Schedule: chunk 0 computes all eight z m-groups first (they need only the
small fp8 operands, ~1.6 MB) while the bf16 x / wh / wo stream in, then
the u groups; later chunks run z groups in quads then their u groups. The
PE pays ~195ns to enter a DoubleRow chain from bf16 (DR->DR group starts
are free), so batching z groups minimizes transitions; four per batch is
the deepest the 5-slot zu PSUM ring and the sigmoid drain rate allow
without micro-stalls (which collapse the PE clock ramp). Output-chunk
matmuls are deferred one chunk so the PE never waits on the serial scan
chain; the final chunk's stores spread over three DMA queues. g lives in
a single [P, MH, TC] buffer reused across chunks (the next chunk's
sigmoid naturally waits for the previous chunk's stt to read its slice).

Sharding: batch B=8 -> one batch element per core; weights broadcast.
"""

import numpy as np
import ml_dtypes

import concourse.bass as bass
import concourse.bacc as bacc
import concourse.mybir as mybir
import concourse.tile as tile
from concourse.bass_utils import run_bass_kernel_spmd
from concourse.bass_interp import get_hw_module
from concourse.tile_rust import add_dep_helper

B, T, I, H, O = 8, 4096, 1024, 1024, 1024
P = 128
TC = 512  # time chunk (matmul free dim / PSUM bank)
SW = 64.0  # fp8 weight scale; the sigmoid ACT divides it out

BF16 = mybir.dt.bfloat16
F32 = mybir.dt.float32
FP8 = mybir.dt.float8e4
NPBF16 = ml_dtypes.bfloat16
NPFP8 = ml_dtypes.float8_e4m3fn

AL = mybir.AluOpType
AF = mybir.ActivationFunctionType
DR = mybir.MatmulPerfMode.DoubleRow


def build_program(t=T, i=I, h=H, o=O, tc_len=TC, n_cores=8, enable_asserts=False):
    KI, MH, MO, NCH = i // P, h // P, o // P, t // tc_len
    KP = i // (2 * P)  # fp8 DoubleRow k-pair count for the z matmul
    nc = bacc.Bacc(
        "TRN2",
        target_bir_lowering=False,
        debug=False,
        enable_asserts=enable_asserts,
        num_devices=n_cores,
    )

    # Host pre-tiled layouts (see kernel() below for the exact packing).
    xT = nc.dram_tensor("xT", [P, NCH, KI, tc_len], BF16, kind="ExternalInput")
    # fp8 moving operand for the z DoubleRow matmuls: planar k-subrow
    # pairs ([p, c, kp, j, t] = x[(2*kp+j)*P+p, c*TC+t]).
    x8d = nc.dram_tensor("x8", [P, NCH, KP, 2, tc_len], FP8, kind="ExternalInput")
    wz8d = nc.dram_tensor("wz8", [P, MH, KP, 2, P], FP8, kind="ExternalInput")
    wh = nc.dram_tensor("wh", [P, MH, KI, P], BF16, kind="ExternalInput")
    wo = nc.dram_tensor("wo", [P, MO, MH, P], BF16, kind="ExternalInput")
    # bz | nbz | bh | bo side by side so one DMA moves all biases.
    biasd = nc.dram_tensor("biases", [P, 3 * MH + MO], F32, kind="ExternalInput")
    out = nc.dram_tensor("out", [P, MO, t], BF16, kind="ExternalOutput")

    with tile.TileContext(nc, pool_alloc_mode="queue") as tcx:
        with (
            tcx.tile_pool(name="weights", bufs=1) as wpool,
            tcx.tile_pool(name="xin", bufs=2) as xpool,
            tcx.tile_pool(name="gtmp", bufs=4) as gpool,
            tcx.tile_pool(name="g0buf", bufs=1) as g0pool,
            tcx.tile_pool(name="ab", bufs=2) as abpool,
            tcx.tile_pool(name="hsb", bufs=3) as hpool,
            tcx.tile_pool(name="osb", bufs=8) as opool,
            tcx.tile_pool(name="zups", bufs=5, space=bass.MemorySpace.PSUM) as zups,
            tcx.tile_pool(name="ops", bufs=3, space=bass.MemorySpace.PSUM) as ops,
        ):
            x_first = xpool.tile([P, KI, tc_len], BF16, tag="x")
            x8_first = xpool.tile([P, KP, 2, tc_len], FP8, tag="x8")
            wz8_s = wpool.tile([P, MH, KP, 2, P], FP8, tag="wz8")
            wh_s = wpool.tile([P, MH, KI, P], BF16, tag="wh")
            wo_s = wpool.tile([P, MO, MH, P], BF16, tag="wo")
            bias_s = wpool.tile([P, 3 * MH + MO], F32, tag="biases")
            bz_s = bias_s[:, 0:MH]
            nbz_s = bias_s[:, MH : 2 * MH]
            bh_s = bias_s[:, 2 * MH : 3 * MH]
            bo_s = bias_s[:, 3 * MH : 3 * MH + MO]

            # Pre-warm the PE while the startup DMAs are in flight: the HAM
            # clock gate needs ~3us of continuous matmul work to reach full
            # speed. The scratch memset runs on the vector engine; the PSUM
            # tile comes from the (idle until ~37us) o ring, never read.
            warm_sb = gpool.tile([P, tc_len], BF16, tag="warm")
            nc.vector.memset(warm_sb[:], 0.0)
            warm_ps = ops.tile([P, tc_len], F32, tag="o")

            def emit_warm(n):
                for _ in range(n):
                    nc.tensor.matmul(
                        warm_ps[:], warm_sb[:, 0:P], warm_sb[:], start=True, stop=True
                    )

            emit_warm(8)

            # Critical startup transfers across three queues, each ordered by
            # consumption. The z phase of chunk 0 needs only x8 + wz8
            # (fp8, ~1.5 MB); x bf16 / wh stream in behind it for the u
            # phase; wo and later chunks are gated on compute progress. The
            # first ~25us are DMA-ramp-bound, so the early window carries
            # only bytes needed before ~25us (~6 MB).
            half = KI // 2
            nc.sync.dma_start(bias_s[:], biasd[:])
            nc.sync.dma_start(wz8_s[:, 0], wz8d[:, 0])
            nc.sync.dma_start(wz8_s[:, 1], wz8d[:, 1])
            for kp in range(KP):
                nc.scalar.dma_start(x8_first[:, kp], x8d[:, 0, kp])
            # wh / x-front wait for the first z matmul: the pre-10us DMA
            # trickle (~58 GB/s) is shared across active queues, so keep it
            # exclusively for the z phase's x8 + wz8.

            def emit_out_chunk(c, h_tile, final=False):
                sl = slice(c * tc_len, (c + 1) * tc_len)
                for mo in range(MO):
                    if final and mo == MO - 1:
                        # The very last output tile is the kernel's critical
                        # tail. Accumulate it as two half-width PSUM groups
                        # with independent consumer chains on separate
                        # engines and DMA queues.
                        hl = tc_len // 2
                        sl_a = slice(c * tc_len, c * tc_len + hl)
                        sl_b = slice(c * tc_len + hl, (c + 1) * tc_len)
                        o_psa = ops.tile([P, tc_len], F32, tag="o")
                        o_psb = ops.tile([P, tc_len], F32, tag="o")
                        for k in range(MH):
                            nc.tensor.matmul(
                                o_psa[:, 0:hl],
                                wo_s[:, mo, k, :],
                                h_tile[:, k, 0:hl],
                                start=(k == 0),
                                stop=(k == MH - 1),
                            )
                            nc.tensor.matmul(
                                o_psb[:, 0:hl],
                                wo_s[:, mo, k, :],
                                h_tile[:, k, hl:tc_len],
                                start=(k == 0),
                                stop=(k == MH - 1),
                            )
                        o_sb = opool.tile([P, tc_len], BF16, tag="osb")
                        nc.vector.tensor_scalar_add(
                            o_sb[:, hl:tc_len], o_psb[:, 0:hl],
                            bo_s[:, mo : mo + 1],
                        )
                        nc.scalar.activation(
                            o_sb[:, 0:hl],
                            o_psa[:, 0:hl],
                            AF.Identity,
                            bias=bo_s[:, mo : mo + 1],
                        )
                        nc.scalar.dma_start(out[:, mo, sl_b], o_sb[:, hl:tc_len])
                        nc.sync.dma_start(out[:, mo, sl_a], o_sb[:, 0:hl])
                        continue
                    o_ps = ops.tile([P, tc_len], F32, tag="o")
                    for k in range(MH):
                        nc.tensor.matmul(
                            o_ps[:],
                            wo_s[:, mo, k, :],
                            h_tile[:, k, :],
                            start=(k == 0),
                            stop=(k == MH - 1),
                        )
                    o_sb = opool.tile([P, tc_len], BF16, tag="osb")
                    # Bias-add on the scalar engine: keeps o-PSUM recycling
                    # off the DVE queue (which carries the scan chain).
                    nc.scalar.activation(
                        o_sb[:], o_ps[:], AF.Identity, bias=bo_s[:, mo : mo + 1]
                    )
                    # Spread the final chunk's stores over three queues so
                    # the end-of-kernel DMA drain parallelizes.
                    q = (nc.sync, nc.scalar, nc.gpsimd)[mo % 3] if final else nc.sync
                    q.dma_start(out[:, mo, sl], o_sb[:])

            def emit_z(m, z_ps, x8_s, c):
                for kp in range(KP):
                    mm = nc.tensor.matmul(
                        z_ps[:],
                        wz8_s[:, m, kp],
                        x8_s[:, kp],
                        start=(kp == 0),
                        stop=(kp == KP - 1),
                        perf_mode=DR,
                    )
                    if c == 0 and m == 0 and kp == 0 and MH > 2:
                        d = nc.sync.dma_start(wz8_s[:, 2:MH], wz8d[:, 2:MH])
                        add_dep_helper(d.ins, mm.ins, True, "wz8 bulk after start")
                        for mw in (0, 1):
                            d = nc.gpsimd.dma_start(wh_s[:, mw], wh[:, mw])
                            add_dep_helper(d.ins, mm.ins, True, "wh early")
                        half0 = KI // 2
                        for kk in range(0, half0, 2):
                            d = nc.gpsimd.dma_start(
                                x_first[:, kk : kk + 2], xT[:, 0, kk : kk + 2]
                            )
                            add_dep_helper(d.ins, mm.ins, True, "x c0 front")
                        for mw in (2, 3):
                            d = nc.gpsimd.dma_start(wh_s[:, mw], wh[:, mw])
                            add_dep_helper(d.ins, mm.ins, True, "wh early")
                return mm

            def emit_u(m, u_ps, x_s):
                for k in range(KI):
                    mm = nc.tensor.matmul(
                        u_ps[:],
                        wh_s[:, m, k, :],
                        x_s[:, k, :],
                        start=(k == 0),
                        stop=(k == KI - 1),
                    )
                return mm

            def emit_gates(m, z_ps, g_dst, a_s):
                # g = sigmoid(z + bz); a = 1 - g = sigmoid(-z - bz)
                nc.scalar.activation(
                    g_dst, z_ps[:], AF.Sigmoid, bias=bz_s[:, m : m + 1],
                    scale=1.0 / SW,
                )
                nc.scalar.activation(
                    a_s[:, m, :], z_ps[:], AF.Sigmoid, bias=nbz_s[:, m : m + 1],
                    scale=-1.0 / SW,
                )

            def emit_scan(m, c, u_ps, g_src, a_s, b_s, h_s, h_prev):
                # b = (u + bh) * g
                nc.vector.scalar_tensor_tensor(
                    b_s[:, m, :], u_ps[:], bh_s[:, m : m + 1], g_src, AL.add, AL.mult
                )
                # h[:, t] = a[:, t] * h[:, t-1] + b[:, t]
                init = 0.0 if c == 0 else h_prev[:, m, tc_len - 1 : tc_len]
                nc.vector.tensor_tensor_scan(
                    h_s[:, m, :], a_s[:, m, :], b_s[:, m, :], init, AL.mult, AL.add
                )

            # ---- chunk 0: z-first phase (fp8-only operands), then u phase.
            a_s = abpool.tile([P, MH, tc_len], F32, tag="a")
            b_s = abpool.tile([P, MH, tc_len], F32, tag="b")
            h_s = hpool.tile([P, MH, tc_len], BF16, tag="h")
            g0 = g0pool.tile([P, MH, tc_len], F32, tag="g0")
            x8_n = None
            for m in range(MH):
                z_ps = zups.tile([P, tc_len], F32, tag="zu")
                mm = emit_z(m, z_ps, x8_first, 0)
                if m == 1:
                    # back half of x chunk 0 behind the x8 loads, and the
                    # remaining u weights.
                    for kk in range(half, KI, 2):
                        d = nc.scalar.dma_start(
                            x_first[:, kk : kk + 2], xT[:, 0, kk : kk + 2]
                        )
                        add_dep_helper(d.ins, mm.ins, True, "x c0 back half")
                    for mw in range(4, MH):
                        d = nc.gpsimd.dma_start(wh_s[:, mw], wh[:, mw])
                        add_dep_helper(d.ins, mm.ins, True, "wh bulk")
                if m == MH - 1 and NCH > 1:
                    # next chunk's fp8 x on the (now idle) sync queue so
                    # chunk 1's z groups aren't starved.
                    x8_n = xpool.tile([P, KP, 2, tc_len], FP8, tag="x8")
                    d = nc.sync.dma_start(x8_n[:], x8d[:, 1])
                    add_dep_helper(d.ins, mm.ins, True, "x8 c1 after z phase")
                emit_gates(m, z_ps, g0[:, m, :], a_s)
            emit_warm(2)
            x_next = None
            for m in range(MH):
                u_ps = zups.tile([P, tc_len], F32, tag="zu")
                mm = emit_u(m, u_ps, x_first)
                if m == 0 and NCH > 1:
                    xb_n = xpool.tile([P, KI, tc_len], BF16, tag="x")
                    d = nc.gpsimd.dma_start(xb_n[:], xT[:, 1])
                    add_dep_helper(d.ins, mm.ins, True, "x c1 after u start")
                    x_next = (xb_n, x8_n)
                if m == 2:
                    d = nc.sync.dma_start(wo_s[:], wo[:])
                    add_dep_helper(d.ins, mm.ins, True, "wo after u start")
                emit_scan(m, 0, u_ps, g0[:, m, :], a_s, b_s, h_s, None)
            h_prev = h_s

            # ---- chunks 1..NCH-1: interleaved z,u per m-group.
            for c in range(1, NCH):
                if x_next is not None:
                    (x_s, x8_s), x_next = x_next, None
                else:
                    x8_s = xpool.tile([P, KP, 2, tc_len], FP8, tag="x8")
                    nc.gpsimd.dma_start(x8_s[:], x8d[:, c])
                    x_s = xpool.tile([P, KI, tc_len], BF16, tag="x")
                    nc.gpsimd.dma_start(x_s[:], xT[:, c])

                a_s = abpool.tile([P, MH, tc_len], F32, tag="a")
                b_s = abpool.tile([P, MH, tc_len], F32, tag="b")
                h_s = hpool.tile([P, MH, tc_len], BF16, tag="h")

                # z groups in quads: the PE pays ~195ns to enter a DR chain
                # from bf16 (DR->DR group starts are free), so batch four z
                # groups per transition. Four is the deepest batch whose
                # PSUM-ring / sigmoid-drain slacks stay ~1us (a full batch
                # of eight collapses the PE clock ramp on micro-stalls).
                for mp in range(0, MH, 4):
                    grp = range(mp, mp + 4)
                    zps_grp = []
                    for m in grp:
                        z_ps = zups.tile([P, tc_len], F32, tag="zu")
                        emit_z(m, z_ps, x8_s, c)
                        zps_grp.append(z_ps)
                        emit_gates(m, z_ps, g0[:, m, :], a_s)
                    for m in grp:
                        u_ps = zups.tile([P, tc_len], F32, tag="zu")
                        emit_u(m, u_ps, x_s)
                        emit_scan(m, c, u_ps, g0[:, m, :], a_s, b_s, h_s, h_prev)

                # Output matmuls for the previous chunk, emitted after this
                # chunk's gate/update matmuls so the PE stream never has to
                # wait on the (serial) scan chain.
                emit_out_chunk(c - 1, h_prev)
                h_prev = h_s
            emit_out_chunk(NCH - 1, h_prev, final=True)

    nc.compile()
    return nc


_CACHED_NC = None


def _get_nc():
    global _CACHED_NC
    if _CACHED_NC is None:
        _CACHED_NC = build_program()
    return _CACHED_NC


# Set by test harnesses that want a profile: kernel() stores the raw
# BassKernelResults of the last run here when TRACE is truthy.
TRACE = False
LAST_RESULTS = None


def _pack_weight(w):
    # [out_dim, in_dim] -> lhsT tiles [P, M_tiles, K_tiles, P] where
    # arr[p, m, k, q] = w[m*P + q, k*P + p]
    kd, md = w.shape[1] // P, w.shape[0] // P
    return np.ascontiguousarray(
        w.T.reshape(kd, P, md, P).transpose(1, 2, 0, 3).astype(NPBF16)
    )


def kernel(**inputs):
    global LAST_RESULTS
    xs = np.asarray(inputs["xs"], np.float32)
    Wz = np.asarray(inputs["Wz"], np.float32)
    bz = np.asarray(inputs["bz"], np.float32)
    Wh = np.asarray(inputs["Wh"], np.float32)
    bh = np.asarray(inputs["bh"], np.float32)
    Wo = np.asarray(inputs["Wo"], np.float32)
    bo = np.asarray(inputs["bo"], np.float32)

    KI, MH, MO, NCH = I // P, H // P, O // P, T // TC
    KP = I // (2 * P)

    # z weights as fp8 DoubleRow tiles, scaled by SW (the sigmoid ACT
    # divides it back out).
    wz8_t = np.ascontiguousarray(
        (Wz * SW).reshape(MH, P, KP, 2, P).transpose(4, 0, 2, 3, 1).astype(NPFP8)
    )
    wh_t = _pack_weight(Wh)
    wo_t = _pack_weight(Wo)
    bias_p = np.concatenate(
        [
            bz.reshape(MH, P).T,
            (-bz).reshape(MH, P).T,
            bh.reshape(MH, P).T,
            bo.reshape(MO, P).T,
        ],
        axis=1,
    )
    bias_p = np.ascontiguousarray(bias_p, np.float32)

    in_maps = []
    for b in range(B):
        # [T, I] -> [P, NCH, KI, TC] with x[p, c, k, t] = xs[b, c*TC+t, k*P+p]
        xb = xs[b].astype(NPBF16).reshape(NCH, TC, KI, P)
        xb = np.ascontiguousarray(xb.transpose(3, 0, 2, 1))
        # fp8 planar k-pair moving operand: [p, c, kp, j, t]
        x8b = np.ascontiguousarray(
            xs[b].reshape(NCH, TC, KP, 2, P).transpose(4, 0, 2, 3, 1).astype(NPFP8)
        )
        in_maps.append(
            {
                "xT": xb,
                "x8": x8b,
                "wz8": wz8_t,
                "wh": wh_t,
                "wo": wo_t,
                "biases": bias_p,
            }
        )

    nc = _get_nc()
    old_m = nc.m
    nc.m = get_hw_module(nc.m)
    try:
        res = run_bass_kernel_spmd(
            nc, in_maps, core_ids=list(range(B)), trace=bool(TRACE)
        )
    finally:
        nc.m = old_m
    LAST_RESULTS = res

    out_full = np.empty((B, T, O), np.float32)
    for b in range(B):
        # [P, MO, T] bf16 -> [O, T] -> [T, O] f32
        ob = np.asarray(res.results[b]["out"], dtype=np.float32)
        out_full[b] = ob.transpose(1, 0, 2).reshape(O, T).T
    return out_full


# revision 20
# speedup vs baseline: 1.0119x; 1.0024x over previous
"""MinGRU layer kernel for Trainium2 (8 NeuronCores, data-parallel over batch).

Math per batch element b (reference semantics):
    z_t = Wz @ x_t + bz ; g_t = sigmoid(z_t)
    u_t = Wh @ x_t + bh
    h_t = (1-g_t) * h_{t-1} + g_t * u_t     (linear recurrence along T)
    y_t = Wo @ h_t + bo
Device layout: hidden dim on partitions (8 tiles x 128), time on the free
dim, chunked by TC=512 columns. The recurrence runs on the DVE
``tensor_tensor_scan`` instruction (state = a*state + b along the free dim)
with a = sigmoid(-z-bz) = 1-g and b = (u+bh)*g.

Precision plan (validated against the reference in fp-exact simulation,
rel_l2 = 1.78e-2 < 2e-2): the z matmul runs entirely in fp8 DoubleRow
(2x PE throughput; the sigmoid's g*(1-g) <= 0.25 slope attenuates the fp8
noise), u and o matmuls in bf16 with fp32 PSUM accumulation (fp8 there
costs ~3.8e-2 rel err - over budget). h is stored bf16; out is stored
bf16 (host converts to f32).

Schedule: chunk 0 computes all eight z m-groups first (they need only the
small fp8 operands, ~1.6 MB) while the bf16 x / wh / wo stream in, then
the u groups; later chunks run z groups in quads then their u groups. The
PE pays ~195ns to enter a DoubleRow chain from bf16 (DR->DR group starts
are free), so batching z groups minimizes transitions; four per batch is
the deepest the 5-slot zu PSUM ring and the sigmoid drain rate allow
without micro-stalls (which collapse the PE clock ramp). Output-chunk
matmuls are deferred one chunk so the PE never waits on the serial scan
chain; the final chunk's stores spread over three DMA queues. g lives in
a single [P, MH, TC] buffer reused across chunks (the next chunk's
sigmoid naturally waits for the previous chunk's stt to read its slice).

Sharding: batch B=8 -> one batch element per core; weights broadcast.
"""

import numpy as np
import ml_dtypes

import concourse.bass as bass
import concourse.bacc as bacc
import concourse.mybir as mybir
import concourse.tile as tile
from concourse.bass_utils import run_bass_kernel_spmd
from concourse.bass_interp import get_hw_module
from concourse.tile_rust import add_dep_helper

B, T, I, H, O = 8, 4096, 1024, 1024, 1024
P = 128
TC = 512  # time chunk (matmul free dim / PSUM bank)
SW = 64.0  # fp8 weight scale; the sigmoid ACT divides it out

BF16 = mybir.dt.bfloat16
F32 = mybir.dt.float32
FP8 = mybir.dt.float8e4
NPBF16 = ml_dtypes.bfloat16
NPFP8 = ml_dtypes.float8_e4m3fn

AL = mybir.AluOpType
AF = mybir.ActivationFunctionType
DR = mybir.MatmulPerfMode.DoubleRow


def build_program(t=T, i=I, h=H, o=O, tc_len=TC, n_cores=8, enable_asserts=False):
    KI, MH, MO, NCH = i // P, h // P, o // P, t // tc_len
    KP = i // (2 * P)  # fp8 DoubleRow k-pair count for the z matmul
    nc = bacc.Bacc(
        "TRN2",
        target_bir_lowering=False,
        debug=False,
        enable_asserts=enable_asserts,
        num_devices=n_cores,
    )

    # Host pre-tiled layouts (see kernel() below for the exact packing).
    xT = nc.dram_tensor("xT", [P, NCH, KI, tc_len], BF16, kind="ExternalInput")
    # fp8 moving operand for the z DoubleRow matmuls: planar k-subrow
    # pairs ([p, c, kp, j, t] = x[(2*kp+j)*P+p, c*TC+t]).
    x8d = nc.dram_tensor("x8", [P, NCH, KP, 2, tc_len], FP8, kind="ExternalInput")
    wz8d = nc.dram_tensor("wz8", [P, MH, KP, 2, P], FP8, kind="ExternalInput")
    wh = nc.dram_tensor("wh", [P, MH, KI, P], BF16, kind="ExternalInput")
    wo = nc.dram_tensor("wo", [P, MO, MH, P], BF16, kind="ExternalInput")
    # bz | nbz | bh | bo side by side so one DMA moves all biases.
    biasd = nc.dram_tensor("biases", [P, 3 * MH + MO], F32, kind="ExternalInput")
    out = nc.dram_tensor("out", [P, MO, t], BF16, kind="ExternalOutput")

    with tile.TileContext(nc, pool_alloc_mode="queue") as tcx:
        with (
            tcx.tile_pool(name="weights", bufs=1) as wpool,
            tcx.tile_pool(name="xin", bufs=2) as xpool,
            tcx.tile_pool(name="gtmp", bufs=4) as gpool,
            tcx.tile_pool(name="g0buf", bufs=1) as g0pool,
            tcx.tile_pool(name="ab", bufs=2) as abpool,
            tcx.tile_pool(name="hsb", bufs=3) as hpool,
            tcx.tile_pool(name="osb", bufs=8) as opool,
            tcx.tile_pool(name="zups", bufs=5, space=bass.MemorySpace.PSUM) as zups,
            tcx.tile_pool(name="ops", bufs=3, space=bass.MemorySpace.PSUM) as ops,
        ):
            x_first = xpool.tile([P, KI, tc_len], BF16, tag="x")
            x8_first = xpool.tile([P, KP, 2, tc_len], FP8, tag="x8")
            wz8_s = wpool.tile([P, MH, KP, 2, P], FP8, tag="wz8")
            wh_s = wpool.tile([P, MH, KI, P], BF16, tag="wh")
            wo_s = wpool.tile([P, MO, MH, P], BF16, tag="wo")
            bias_s = wpool.tile([P, 3 * MH + MO], F32, tag="biases")
            bz_s = bias_s[:, 0:MH]
            nbz_s = bias_s[:, MH : 2 * MH]
            bh_s = bias_s[:, 2 * MH : 3 * MH]
            bo_s = bias_s[:, 3 * MH : 3 * MH + MO]

            # Pre-warm the PE while the startup DMAs are in flight: the HAM
            # clock gate needs ~3us of continuous matmul work to reach full
            # speed. The scratch memset runs on the vector engine; the PSUM
            # tile comes from the (idle until ~37us) o ring, never read.
            warm_sb = gpool.tile([P, tc_len], BF16, tag="warm")
            nc.vector.memset(warm_sb[:], 0.0)
            warm_ps = ops.tile([P, tc_len], F32, tag="o")

            def emit_warm(n):
                for _ in range(n):
                    nc.tensor.matmul(
                        warm_ps[:], warm_sb[:, 0:P], warm_sb[:], start=True, stop=True
                    )

            emit_warm(8)

            # Critical startup transfers across three queues, each ordered by
            # consumption. The z phase of chunk 0 needs only x8 + wz8
            # (fp8, ~1.5 MB); x bf16 / wh stream in behind it for the u
            # phase; wo and later chunks are gated on compute progress. The
            # first ~25us are DMA-ramp-bound, so the early window carries
            # only bytes needed before ~25us (~6 MB).
            half = KI // 2
            nc.sync.dma_start(bias_s[:], biasd[:])
            nc.sync.dma_start(wz8_s[:, 0], wz8d[:, 0])
            nc.sync.dma_start(wz8_s[:, 1], wz8d[:, 1])
            nc.scalar.dma_start(x8_first[:, 0], x8d[:, 0, 0])
            nc.scalar.dma_start(x8_first[:, 1], x8d[:, 0, 1])
            nc.gpsimd.dma_start(x8_first[:, 2], x8d[:, 0, 2])
            nc.gpsimd.dma_start(x8_first[:, 3], x8d[:, 0, 3])
            # wh / x-front wait for the first z matmul: the pre-10us DMA
            # trickle (~58 GB/s) is shared across active queues, so keep it
            # exclusively for the z phase's x8 + wz8.

            def emit_out_chunk(c, h_tile, final=False):
                sl = slice(c * tc_len, (c + 1) * tc_len)
                for mo in range(MO):
                    if final and mo == MO - 1:
                        # The very last output tile is the kernel's critical
                        # tail. Accumulate it as two half-width PSUM groups
                        # with independent consumer chains on separate
                        # engines and DMA queues.
                        hl = tc_len // 2
                        sl_a = slice(c * tc_len, c * tc_len + hl)
                        sl_b = slice(c * tc_len + hl, (c + 1) * tc_len)
                        o_psa = ops.tile([P, tc_len], F32, tag="o")
                        o_psb = ops.tile([P, tc_len], F32, tag="o")
                        for k in range(MH):
                            nc.tensor.matmul(
                                o_psa[:, 0:hl],
                                wo_s[:, mo, k, :],
                                h_tile[:, k, 0:hl],
                                start=(k == 0),
                                stop=(k == MH - 1),
                            )
                            nc.tensor.matmul(
                                o_psb[:, 0:hl],
                                wo_s[:, mo, k, :],
                                h_tile[:, k, hl:tc_len],
                                start=(k == 0),
                                stop=(k == MH - 1),
                            )
                        o_sb = opool.tile([P, tc_len], BF16, tag="osb")
                        nc.vector.tensor_scalar_add(
                            o_sb[:, hl:tc_len], o_psb[:, 0:hl],
                            bo_s[:, mo : mo + 1],
                        )
                        nc.scalar.activation(
                            o_sb[:, 0:hl],
                            o_psa[:, 0:hl],
                            AF.Identity,
                            bias=bo_s[:, mo : mo + 1],
                        )
                        nc.scalar.dma_start(out[:, mo, sl_b], o_sb[:, hl:tc_len])
                        nc.sync.dma_start(out[:, mo, sl_a], o_sb[:, 0:hl])
                        continue
                    o_ps = ops.tile([P, tc_len], F32, tag="o")
                    for k in range(MH):
                        nc.tensor.matmul(
                            o_ps[:],
                            wo_s[:, mo, k, :],
                            h_tile[:, k, :],
                            start=(k == 0),
                            stop=(k == MH - 1),
                        )
                    o_sb = opool.tile([P, tc_len], BF16, tag="osb")
                    # Bias-add on the scalar engine: keeps o-PSUM recycling
                    # off the DVE queue (which carries the scan chain).
                    nc.scalar.activation(
                        o_sb[:], o_ps[:], AF.Identity, bias=bo_s[:, mo : mo + 1]
                    )
                    # Spread the final chunk's stores over three queues so
                    # the end-of-kernel DMA drain parallelizes.
                    q = (nc.sync, nc.scalar, nc.gpsimd)[mo % 3] if final else nc.sync
                    q.dma_start(out[:, mo, sl], o_sb[:])

            def emit_z(m, z_ps, x8_s, c):
                for kp in range(KP):
                    mm = nc.tensor.matmul(
                        z_ps[:],
                        wz8_s[:, m, kp],
                        x8_s[:, kp],
                        start=(kp == 0),
                        stop=(kp == KP - 1),
                        perf_mode=DR,
                    )
                    if c == 0 and m == 0 and kp == 0 and MH > 2:
                        d = nc.sync.dma_start(wz8_s[:, 2:MH], wz8d[:, 2:MH])
                        add_dep_helper(d.ins, mm.ins, True, "wz8 bulk after start")
                        for mw in (0, 1):
                            d = nc.gpsimd.dma_start(wh_s[:, mw], wh[:, mw])
                            add_dep_helper(d.ins, mm.ins, True, "wh early")
                        half0 = KI // 2
                        for kk in range(0, half0, 2):
                            d = nc.gpsimd.dma_start(
                                x_first[:, kk : kk + 2], xT[:, 0, kk : kk + 2]
                            )
                            add_dep_helper(d.ins, mm.ins, True, "x c0 front")
                        for mw in (2, 3):
                            d = nc.gpsimd.dma_start(wh_s[:, mw], wh[:, mw])
                            add_dep_helper(d.ins, mm.ins, True, "wh early")
                return mm

            def emit_u(m, u_ps, x_s):
                for k in range(KI):
                    mm = nc.tensor.matmul(
                        u_ps[:],
                        wh_s[:, m, k, :],
                        x_s[:, k, :],
                        start=(k == 0),
                        stop=(k == KI - 1),
                    )
                return mm

            def emit_gates(m, z_ps, g_dst, a_s):
                # g = sigmoid(z + bz); a = 1 - g = sigmoid(-z - bz)
                nc.scalar.activation(
                    g_dst, z_ps[:], AF.Sigmoid, bias=bz_s[:, m : m + 1],
                    scale=1.0 / SW,
                )
                nc.scalar.activation(
                    a_s[:, m, :], z_ps[:], AF.Sigmoid, bias=nbz_s[:, m : m + 1],
                    scale=-1.0 / SW,
                )

            def emit_scan(m, c, u_ps, g_src, a_s, b_s, h_s, h_prev):
                # b = (u + bh) * g
                nc.vector.scalar_tensor_tensor(
                    b_s[:, m, :], u_ps[:], bh_s[:, m : m + 1], g_src, AL.add, AL.mult
                )
                # h[:, t] = a[:, t] * h[:, t-1] + b[:, t]
                init = 0.0 if c == 0 else h_prev[:, m, tc_len - 1 : tc_len]
                nc.vector.tensor_tensor_scan(
                    h_s[:, m, :], a_s[:, m, :], b_s[:, m, :], init, AL.mult, AL.add
                )

            # ---- chunk 0: z-first phase (fp8-only operands), then u phase.
            a_s = abpool.tile([P, MH, tc_len], F32, tag="a")
            b_s = abpool.tile([P, MH, tc_len], F32, tag="b")
            h_s = hpool.tile([P, MH, tc_len], BF16, tag="h")
            g0 = g0pool.tile([P, MH, tc_len], F32, tag="g0")
            x8_n = None
            for m in range(MH):
                z_ps = zups.tile([P, tc_len], F32, tag="zu")
                mm = emit_z(m, z_ps, x8_first, 0)
                if m == 1:
                    # back half of x chunk 0 behind the x8 loads, and the
                    # remaining u weights.
                    for kk in range(half, KI, 2):
                        d = nc.scalar.dma_start(
                            x_first[:, kk : kk + 2], xT[:, 0, kk : kk + 2]
                        )
                        add_dep_helper(d.ins, mm.ins, True, "x c0 back half")
                    for mw in range(4, MH):
                        d = nc.gpsimd.dma_start(wh_s[:, mw], wh[:, mw])
                        add_dep_helper(d.ins, mm.ins, True, "wh bulk")
                if m == MH - 1 and NCH > 1:
                    # next chunk's fp8 x on the (now idle) sync queue so
                    # chunk 1's z groups aren't starved.
                    x8_n = xpool.tile([P, KP, 2, tc_len], FP8, tag="x8")
                    d = nc.sync.dma_start(x8_n[:], x8d[:, 1])
                    add_dep_helper(d.ins, mm.ins, True, "x8 c1 after z phase")
                emit_gates(m, z_ps, g0[:, m, :], a_s)
            emit_warm(2)
            x_next = None
            for m in range(MH):
                u_ps = zups.tile([P, tc_len], F32, tag="zu")
                mm = emit_u(m, u_ps, x_first)
                if m == 0 and NCH > 1:
                    xb_n = xpool.tile([P, KI, tc_len], BF16, tag="x")
                    d = nc.gpsimd.dma_start(xb_n[:], xT[:, 1])
                    add_dep_helper(d.ins, mm.ins, True, "x c1 after u start")
                    x_next = (xb_n, x8_n)
                if m == 2:
                    d = nc.sync.dma_start(wo_s[:], wo[:])
                    add_dep_helper(d.ins, mm.ins, True, "wo after u start")
                emit_scan(m, 0, u_ps, g0[:, m, :], a_s, b_s, h_s, None)
            h_prev = h_s

            # ---- chunks 1..NCH-1: interleaved z,u per m-group.
            for c in range(1, NCH):
                if x_next is not None:
                    (x_s, x8_s), x_next = x_next, None
                else:
                    x8_s = xpool.tile([P, KP, 2, tc_len], FP8, tag="x8")
                    nc.gpsimd.dma_start(x8_s[:], x8d[:, c])
                    x_s = xpool.tile([P, KI, tc_len], BF16, tag="x")
                    nc.gpsimd.dma_start(x_s[:], xT[:, c])

                a_s = abpool.tile([P, MH, tc_len], F32, tag="a")
                b_s = abpool.tile([P, MH, tc_len], F32, tag="b")
                h_s = hpool.tile([P, MH, tc_len], BF16, tag="h")

                # z groups in quads: the PE pays ~195ns to enter a DR chain
                # from bf16 (DR->DR group starts are free), so batch four z
                # groups per transition. Four is the deepest batch whose
                # PSUM-ring / sigmoid-drain slacks stay ~1us (a full batch
                # of eight collapses the PE clock ramp on micro-stalls).
                for mp in range(0, MH, 4):
                    grp = range(mp, mp + 4)
                    zps_grp = []
                    for m in grp:
                        z_ps = zups.tile([P, tc_len], F32, tag="zu")
                        emit_z(m, z_ps, x8_s, c)
                        zps_grp.append(z_ps)
                        emit_gates(m, z_ps, g0[:, m, :], a_s)
                    for m in grp:
                        u_ps = zups.tile([P, tc_len], F32, tag="zu")
                        emit_u(m, u_ps, x_s)
                        emit_scan(m, c, u_ps, g0[:, m, :], a_s, b_s, h_s, h_prev)

                # Output matmuls for the previous chunk, emitted after this
                # chunk's gate/update matmuls so the PE stream never has to
                # wait on the (serial) scan chain.
                emit_out_chunk(c - 1, h_prev)
                h_prev = h_s
            emit_out_chunk(NCH - 1, h_prev, final=True)

    nc.compile()
    return nc


_CACHED_NC = None


def _get_nc():
    global _CACHED_NC
    if _CACHED_NC is None:
        _CACHED_NC = build_program()
    return _CACHED_NC


# Set by test harnesses that want a profile: kernel() stores the raw
# BassKernelResults of the last run here when TRACE is truthy.
TRACE = False
LAST_RESULTS = None


def _pack_weight(w):
    # [out_dim, in_dim] -> lhsT tiles [P, M_tiles, K_tiles, P] where
    # arr[p, m, k, q] = w[m*P + q, k*P + p]
    kd, md = w.shape[1] // P, w.shape[0] // P
    return np.ascontiguousarray(
        w.T.reshape(kd, P, md, P).transpose(1, 2, 0, 3).astype(NPBF16)
    )


def kernel(**inputs):
    global LAST_RESULTS
    xs = np.asarray(inputs["xs"], np.float32)
    Wz = np.asarray(inputs["Wz"], np.float32)
    bz = np.asarray(inputs["bz"], np.float32)
    Wh = np.asarray(inputs["Wh"], np.float32)
    bh = np.asarray(inputs["bh"], np.float32)
    Wo = np.asarray(inputs["Wo"], np.float32)
    bo = np.asarray(inputs["bo"], np.float32)

    KI, MH, MO, NCH = I // P, H // P, O // P, T // TC
    KP = I // (2 * P)

    # z weights as fp8 DoubleRow tiles, scaled by SW (the sigmoid ACT
    # divides it back out).
    wz8_t = np.ascontiguousarray(
        (Wz * SW).reshape(MH, P, KP, 2, P).transpose(4, 0, 2, 3, 1).astype(NPFP8)
    )
    wh_t = _pack_weight(Wh)
    wo_t = _pack_weight(Wo)
    bias_p = np.concatenate(
        [
            bz.reshape(MH, P).T,
            (-bz).reshape(MH, P).T,
            bh.reshape(MH, P).T,
            bo.reshape(MO, P).T,
        ],
        axis=1,
    )
    bias_p = np.ascontiguousarray(bias_p, np.float32)

    in_maps = []
    for b in range(B):
        # [T, I] -> [P, NCH, KI, TC] with x[p, c, k, t] = xs[b, c*TC+t, k*P+p]
        xb = xs[b].astype(NPBF16).reshape(NCH, TC, KI, P)
        xb = np.ascontiguousarray(xb.transpose(3, 0, 2, 1))
        # fp8 planar k-pair moving operand: [p, c, kp, j, t]
        x8b = np.ascontiguousarray(
            xs[b].reshape(NCH, TC, KP, 2, P).transpose(4, 0, 2, 3, 1).astype(NPFP8)
        )
        in_maps.append(
            {
                "xT": xb,
                "x8": x8b,
                "wz8": wz8_t,
                "wh": wh_t,
                "wo": wo_t,
                "biases": bias_p,
            }
        )

    nc = _get_nc()
    old_m = nc.m
    nc.m = get_hw_module(nc.m)
    try:
        res = run_bass_kernel_spmd(
            nc, in_maps, core_ids=list(range(B)), trace=bool(TRACE)
        )
    finally:
        nc.m = old_m
    LAST_RESULTS = res

    out_full = np.empty((B, T, O), np.float32)
    for b in range(B):
        # [P, MO, T] bf16 -> [O, T] -> [T, O] f32
        ob = np.asarray(res.results[b]["out"], dtype=np.float32)
        out_full[b] = ob.transpose(1, 0, 2).reshape(O, T).T
    return out_full


# revision 21
# speedup vs baseline: 1.0209x; 1.0089x over previous
"""MinGRU layer kernel for Trainium2 (8 NeuronCores, data-parallel over batch).

Math per batch element b (reference semantics):
    z_t = Wz @ x_t + bz ; g_t = sigmoid(z_t)
    u_t = Wh @ x_t + bh
    h_t = (1-g_t) * h_{t-1} + g_t * u_t     (linear recurrence along T)
    y_t = Wo @ h_t + bo
Device layout: hidden dim on partitions (8 tiles x 128), time on the free
dim, chunked by TC=512 columns. The recurrence runs on the DVE
``tensor_tensor_scan`` instruction (state = a*state + b along the free dim)
with a = sigmoid(-z-bz) = 1-g and b = (u+bh)*g.

Precision plan (validated against the reference in fp-exact simulation,
rel_l2 = 1.78e-2 < 2e-2): the z matmul runs entirely in fp8 DoubleRow
(2x PE throughput; the sigmoid's g*(1-g) <= 0.25 slope attenuates the fp8
noise), u and o matmuls in bf16 with fp32 PSUM accumulation (fp8 there
costs ~3.8e-2 rel err - over budget). h is stored bf16; out is stored
bf16 (host converts to f32).

Schedule: chunk 0 computes all eight z m-groups first (they need only the
small fp8 operands, ~1.6 MB) while the bf16 x / wh / wo stream in, then
the u groups; later chunks run z groups in quads then their u groups. The
PE pays ~195ns to enter a DoubleRow chain from bf16 (DR->DR group starts
are free), so batching z groups minimizes transitions; four per batch is
the deepest the 5-slot zu PSUM ring and the sigmoid drain rate allow
without micro-stalls (which collapse the PE clock ramp). Output-chunk
matmuls are deferred one chunk so the PE never waits on the serial scan
chain; the final chunk's stores spread over three DMA queues. g lives in
a single [P, MH, TC] buffer reused across chunks (the next chunk's
sigmoid naturally waits for the previous chunk's stt to read its slice).

Sharding: batch B=8 -> one batch element per core; weights broadcast.
"""

import numpy as np
import ml_dtypes

import concourse.bass as bass
import concourse.bacc as bacc
import concourse.mybir as mybir
import concourse.tile as tile
from concourse.bass_utils import run_bass_kernel_spmd
from concourse.bass_interp import get_hw_module
from concourse.tile_rust import add_dep_helper

B, T, I, H, O = 8, 4096, 1024, 1024, 1024
P = 128
TC = 512  # time chunk (matmul free dim / PSUM bank)
SW = 64.0  # fp8 weight scale; the sigmoid ACT divides it out

BF16 = mybir.dt.bfloat16
F32 = mybir.dt.float32
FP8 = mybir.dt.float8e4
NPBF16 = ml_dtypes.bfloat16
NPFP8 = ml_dtypes.float8_e4m3fn

AL = mybir.AluOpType
AF = mybir.ActivationFunctionType
DR = mybir.MatmulPerfMode.DoubleRow


def build_program(t=T, i=I, h=H, o=O, tc_len=TC, n_cores=8, enable_asserts=False):
    KI, MH, MO, NCH = i // P, h // P, o // P, t // tc_len
    KP = i // (2 * P)  # fp8 DoubleRow k-pair count for the z matmul
    nc = bacc.Bacc(
        "TRN2",
        target_bir_lowering=False,
        debug=False,
        enable_asserts=enable_asserts,
        num_devices=n_cores,
    )

    # Host pre-tiled layouts (see kernel() below for the exact packing).
    xT = nc.dram_tensor("xT", [P, NCH, KI, tc_len], BF16, kind="ExternalInput")
    # fp8 moving operand for the z DoubleRow matmuls: planar k-subrow
    # pairs ([p, c, kp, j, t] = x[(2*kp+j)*P+p, c*TC+t]).
    x8d = nc.dram_tensor("x8", [P, NCH, KP, 2, tc_len], FP8, kind="ExternalInput")
    wz8d = nc.dram_tensor("wz8", [P, MH, KP, 2, P], FP8, kind="ExternalInput")
    wh = nc.dram_tensor("wh", [P, MH, KI, P], BF16, kind="ExternalInput")
    wo = nc.dram_tensor("wo", [P, MO, MH, P], BF16, kind="ExternalInput")
    # bz | nbz | bh | bo side by side so one DMA moves all biases.
    biasd = nc.dram_tensor("biases", [P, 3 * MH + MO], F32, kind="ExternalInput")
    out = nc.dram_tensor("out", [P, MO, t], BF16, kind="ExternalOutput")

    with tile.TileContext(nc, pool_alloc_mode="queue") as tcx:
        with (
            tcx.tile_pool(name="weights", bufs=1) as wpool,
            tcx.tile_pool(name="xin", bufs=2) as xpool,
            tcx.tile_pool(name="gtmp", bufs=4) as gpool,
            tcx.tile_pool(name="g0buf", bufs=1) as g0pool,
            tcx.tile_pool(name="ab", bufs=2) as abpool,
            tcx.tile_pool(name="hsb", bufs=3) as hpool,
            tcx.tile_pool(name="osb", bufs=8) as opool,
            tcx.tile_pool(name="zups", bufs=5, space=bass.MemorySpace.PSUM) as zups,
            tcx.tile_pool(name="ops", bufs=3, space=bass.MemorySpace.PSUM) as ops,
        ):
            x_first = xpool.tile([P, KI, tc_len], BF16, tag="x")
            x8_first = xpool.tile([P, KP, 2, tc_len], FP8, tag="x8")
            wz8_s = wpool.tile([P, MH, KP, 2, P], FP8, tag="wz8")
            wh_s = wpool.tile([P, MH, KI, P], BF16, tag="wh")
            wo_s = wpool.tile([P, MO, MH, P], BF16, tag="wo")
            bias_s = wpool.tile([P, 3 * MH + MO], F32, tag="biases")
            bz_s = bias_s[:, 0:MH]
            nbz_s = bias_s[:, MH : 2 * MH]
            bh_s = bias_s[:, 2 * MH : 3 * MH]
            bo_s = bias_s[:, 3 * MH : 3 * MH + MO]

            # Pre-warm the PE while the startup DMAs are in flight: the HAM
            # clock gate needs ~3us of continuous matmul work to reach full
            # speed. The scratch memset runs on the vector engine; the PSUM
            # tile comes from the (idle until ~37us) o ring, never read.
            warm_sb = gpool.tile([P, tc_len], BF16, tag="warm")
            nc.vector.memset(warm_sb[:], 0.0)
            warm_ps = ops.tile([P, tc_len], F32, tag="o")

            def emit_warm(n):
                for _ in range(n):
                    nc.tensor.matmul(
                        warm_ps[:], warm_sb[:, 0:P], warm_sb[:], start=True, stop=True
                    )

            emit_warm(8)

            # Critical startup transfers across three queues, each ordered by
            # consumption. The z phase of chunk 0 needs only x8 + wz8
            # (fp8, ~1.5 MB); x bf16 / wh stream in behind it for the u
            # phase; wo and later chunks are gated on compute progress. The
            # first ~25us are DMA-ramp-bound, so the early window carries
            # only bytes needed before ~25us (~6 MB).
            half = KI // 2
            nc.sync.dma_start(bias_s[:], biasd[:])
            nc.sync.dma_start(wz8_s[:, 0], wz8d[:, 0])
            nc.sync.dma_start(wz8_s[:, 1], wz8d[:, 1])
            nc.scalar.dma_start(x8_first[:, 0], x8d[:, 0, 0])
            nc.scalar.dma_start(x8_first[:, 1], x8d[:, 0, 1])
            nc.gpsimd.dma_start(x8_first[:, 2], x8d[:, 0, 2])
            nc.gpsimd.dma_start(x8_first[:, 3], x8d[:, 0, 3])
            # wh / x-front wait for the first z matmul: the pre-10us DMA
            # trickle (~58 GB/s) is shared across active queues, so keep it
            # exclusively for the z phase's x8 + wz8.

            def emit_out_chunk(c, h_tile, final=False):
                sl = slice(c * tc_len, (c + 1) * tc_len)
                for mo in range(MO):
                    if final and mo == MO - 1:
                        # The very last output tile is the kernel's critical
                        # tail. Accumulate it as two half-width PSUM groups
                        # with independent consumer chains on separate
                        # engines and DMA queues.
                        hl = tc_len // 2
                        sl_a = slice(c * tc_len, c * tc_len + hl)
                        sl_b = slice(c * tc_len + hl, (c + 1) * tc_len)
                        o_psa = ops.tile([P, tc_len], F32, tag="o")
                        o_psb = ops.tile([P, tc_len], F32, tag="o")
                        for k in range(MH):
                            nc.tensor.matmul(
                                o_psa[:, 0:hl],
                                wo_s[:, mo, k, :],
                                h_tile[:, k, 0:hl],
                                start=(k == 0),
                                stop=(k == MH - 1),
                            )
                            nc.tensor.matmul(
                                o_psb[:, 0:hl],
                                wo_s[:, mo, k, :],
                                h_tile[:, k, hl:tc_len],
                                start=(k == 0),
                                stop=(k == MH - 1),
                            )
                        o_sb = opool.tile([P, tc_len], BF16, tag="osb")
                        nc.vector.tensor_scalar_add(
                            o_sb[:, hl:tc_len], o_psb[:, 0:hl],
                            bo_s[:, mo : mo + 1],
                        )
                        nc.scalar.activation(
                            o_sb[:, 0:hl],
                            o_psa[:, 0:hl],
                            AF.Identity,
                            bias=bo_s[:, mo : mo + 1],
                        )
                        nc.scalar.dma_start(out[:, mo, sl_b], o_sb[:, hl:tc_len])
                        nc.sync.dma_start(out[:, mo, sl_a], o_sb[:, 0:hl])
                        continue
                    o_ps = ops.tile([P, tc_len], F32, tag="o")
                    for k in range(MH):
                        nc.tensor.matmul(
                            o_ps[:],
                            wo_s[:, mo, k, :],
                            h_tile[:, k, :],
                            start=(k == 0),
                            stop=(k == MH - 1),
                        )
                    o_sb = opool.tile([P, tc_len], BF16, tag="osb")
                    # Bias-add on the scalar engine: keeps o-PSUM recycling
                    # off the DVE queue (which carries the scan chain).
                    nc.scalar.activation(
                        o_sb[:], o_ps[:], AF.Identity, bias=bo_s[:, mo : mo + 1]
                    )
                    # Spread the final chunk's stores over three queues so
                    # the end-of-kernel DMA drain parallelizes.
                    q = (nc.sync, nc.scalar, nc.gpsimd)[mo % 3] if final else nc.sync
                    q.dma_start(out[:, mo, sl], o_sb[:])

            def emit_z(m, z_ps, x8_s, c):
                for kp in range(KP):
                    mm = nc.tensor.matmul(
                        z_ps[:],
                        wz8_s[:, m, kp],
                        x8_s[:, kp],
                        start=(kp == 0),
                        stop=(kp == KP - 1),
                        perf_mode=DR,
                    )
                    if c == 0 and m == 0 and kp == 0 and MH > 2:
                        d = nc.sync.dma_start(wz8_s[:, 2:MH], wz8d[:, 2:MH])
                        add_dep_helper(d.ins, mm.ins, True, "wz8 bulk after start")
                        for mw in (0, 1):
                            d = nc.gpsimd.dma_start(wh_s[:, mw], wh[:, mw])
                            add_dep_helper(d.ins, mm.ins, True, "wh early")
                        half0 = KI // 2
                        for kk in range(0, half0, 2):
                            d = nc.gpsimd.dma_start(
                                x_first[:, kk : kk + 2], xT[:, 0, kk : kk + 2]
                            )
                            add_dep_helper(d.ins, mm.ins, True, "x c0 front")
                        for mw in (2, 3):
                            d = nc.gpsimd.dma_start(wh_s[:, mw], wh[:, mw])
                            add_dep_helper(d.ins, mm.ins, True, "wh early")
                return mm

            def emit_u(m, u_ps, x_s):
                for k in range(KI):
                    mm = nc.tensor.matmul(
                        u_ps[:],
                        wh_s[:, m, k, :],
                        x_s[:, k, :],
                        start=(k == 0),
                        stop=(k == KI - 1),
                    )
                return mm

            def emit_gates(m, z_ps, g_dst, a_s):
                # g = sigmoid(z + bz); a = 1 - g = sigmoid(-z - bz)
                nc.scalar.activation(
                    g_dst, z_ps[:], AF.Sigmoid, bias=bz_s[:, m : m + 1],
                    scale=1.0 / SW,
                )
                nc.scalar.activation(
                    a_s[:, m, :], z_ps[:], AF.Sigmoid, bias=nbz_s[:, m : m + 1],
                    scale=-1.0 / SW,
                )

            def emit_scan(m, c, u_ps, g_src, a_s, b_s, h_s, h_prev):
                # b = (u + bh) * g
                nc.vector.scalar_tensor_tensor(
                    b_s[:, m, :], u_ps[:], bh_s[:, m : m + 1], g_src, AL.add, AL.mult
                )
                # h[:, t] = a[:, t] * h[:, t-1] + b[:, t]
                init = 0.0 if c == 0 else h_prev[:, m, tc_len - 1 : tc_len]
                nc.vector.tensor_tensor_scan(
                    h_s[:, m, :], a_s[:, m, :], b_s[:, m, :], init, AL.mult, AL.add
                )

            # ---- chunk 0: z-first phase (fp8-only operands), then u phase.
            a_s = abpool.tile([P, MH, tc_len], F32, tag="a")
            b_s = abpool.tile([P, MH, tc_len], F32, tag="b")
            h_s = hpool.tile([P, MH, tc_len], BF16, tag="h")
            g0 = g0pool.tile([P, MH, tc_len], F32, tag="g0")
            x8_n = None
            for m in range(MH):
                z_ps = zups.tile([P, tc_len], F32, tag="zu")
                mm = emit_z(m, z_ps, x8_first, 0)
                if m == 0:
                    # back half of x chunk 0, split across the sync and
                    # scalar queues behind their fp8 loads.
                    for n, kk in enumerate(range(half, KI, 2)):
                        q = nc.sync if n % 2 == 0 else nc.scalar
                        d = q.dma_start(
                            x_first[:, kk : kk + 2], xT[:, 0, kk : kk + 2]
                        )
                        add_dep_helper(d.ins, mm.ins, True, "x c0 back half")
                if m == 1:
                    for mw in range(4, MH):
                        d = nc.gpsimd.dma_start(wh_s[:, mw], wh[:, mw])
                        add_dep_helper(d.ins, mm.ins, True, "wh bulk")
                if m == MH - 1 and NCH > 1:
                    # next chunk's fp8 x on the (now idle) sync queue so
                    # chunk 1's z groups aren't starved.
                    x8_n = xpool.tile([P, KP, 2, tc_len], FP8, tag="x8")
                    d = nc.sync.dma_start(x8_n[:], x8d[:, 1])
                    add_dep_helper(d.ins, mm.ins, True, "x8 c1 after z phase")
                emit_gates(m, z_ps, g0[:, m, :], a_s)
            emit_warm(2)
            x_next = None
            for m in range(MH):
                u_ps = zups.tile([P, tc_len], F32, tag="zu")
                mm = emit_u(m, u_ps, x_first)
                if m == 0 and NCH > 1:
                    xb_n = xpool.tile([P, KI, tc_len], BF16, tag="x")
                    d = nc.gpsimd.dma_start(xb_n[:], xT[:, 1])
                    add_dep_helper(d.ins, mm.ins, True, "x c1 after u start")
                    x_next = (xb_n, x8_n)
                if m == 2:
                    d = nc.sync.dma_start(wo_s[:], wo[:])
                    add_dep_helper(d.ins, mm.ins, True, "wo after u start")
                emit_scan(m, 0, u_ps, g0[:, m, :], a_s, b_s, h_s, None)
            h_prev = h_s

            # ---- chunks 1..NCH-1: interleaved z,u per m-group.
            for c in range(1, NCH):
                if x_next is not None:
                    (x_s, x8_s), x_next = x_next, None
                else:
                    x8_s = xpool.tile([P, KP, 2, tc_len], FP8, tag="x8")
                    nc.gpsimd.dma_start(x8_s[:], x8d[:, c])
                    x_s = xpool.tile([P, KI, tc_len], BF16, tag="x")
                    nc.gpsimd.dma_start(x_s[:], xT[:, c])

                a_s = abpool.tile([P, MH, tc_len], F32, tag="a")
                b_s = abpool.tile([P, MH, tc_len], F32, tag="b")
                h_s = hpool.tile([P, MH, tc_len], BF16, tag="h")

                # z groups in quads: the PE pays ~195ns to enter a DR chain
                # from bf16 (DR->DR group starts are free), so batch four z
                # groups per transition. Four is the deepest batch whose
                # PSUM-ring / sigmoid-drain slacks stay ~1us (a full batch
                # of eight collapses the PE clock ramp on micro-stalls).
                for mp in range(0, MH, 4):
                    grp = range(mp, mp + 4)
                    zps_grp = []
                    for m in grp:
                        z_ps = zups.tile([P, tc_len], F32, tag="zu")
                        emit_z(m, z_ps, x8_s, c)
                        zps_grp.append(z_ps)
                        emit_gates(m, z_ps, g0[:, m, :], a_s)
                    for m in grp:
                        u_ps = zups.tile([P, tc_len], F32, tag="zu")
                        emit_u(m, u_ps, x_s)
                        emit_scan(m, c, u_ps, g0[:, m, :], a_s, b_s, h_s, h_prev)

                # Output matmuls for the previous chunk, emitted after this
                # chunk's gate/update matmuls so the PE stream never has to
                # wait on the (serial) scan chain.
                emit_out_chunk(c - 1, h_prev)
                h_prev = h_s
            emit_out_chunk(NCH - 1, h_prev, final=True)

    nc.compile()
    return nc


_CACHED_NC = None


def _get_nc():
    global _CACHED_NC
    if _CACHED_NC is None:
        _CACHED_NC = build_program()
    return _CACHED_NC


# Set by test harnesses that want a profile: kernel() stores the raw
# BassKernelResults of the last run here when TRACE is truthy.
TRACE = False
LAST_RESULTS = None


def _pack_weight(w):
    # [out_dim, in_dim] -> lhsT tiles [P, M_tiles, K_tiles, P] where
    # arr[p, m, k, q] = w[m*P + q, k*P + p]
    kd, md = w.shape[1] // P, w.shape[0] // P
    return np.ascontiguousarray(
        w.T.reshape(kd, P, md, P).transpose(1, 2, 0, 3).astype(NPBF16)
    )


def kernel(**inputs):
    global LAST_RESULTS
    xs = np.asarray(inputs["xs"], np.float32)
    Wz = np.asarray(inputs["Wz"], np.float32)
    bz = np.asarray(inputs["bz"], np.float32)
    Wh = np.asarray(inputs["Wh"], np.float32)
    bh = np.asarray(inputs["bh"], np.float32)
    Wo = np.asarray(inputs["Wo"], np.float32)
    bo = np.asarray(inputs["bo"], np.float32)

    KI, MH, MO, NCH = I // P, H // P, O // P, T // TC
    KP = I // (2 * P)

    # z weights as fp8 DoubleRow tiles, scaled by SW (the sigmoid ACT
    # divides it back out).
    wz8_t = np.ascontiguousarray(
        (Wz * SW).reshape(MH, P, KP, 2, P).transpose(4, 0, 2, 3, 1).astype(NPFP8)
    )
    wh_t = _pack_weight(Wh)
    wo_t = _pack_weight(Wo)
    bias_p = np.concatenate(
        [
            bz.reshape(MH, P).T,
            (-bz).reshape(MH, P).T,
            bh.reshape(MH, P).T,
            bo.reshape(MO, P).T,
        ],
        axis=1,
    )
    bias_p = np.ascontiguousarray(bias_p, np.float32)

    in_maps = []
    for b in range(B):
        # [T, I] -> [P, NCH, KI, TC] with x[p, c, k, t] = xs[b, c*TC+t, k*P+p]
        xb = xs[b].astype(NPBF16).reshape(NCH, TC, KI, P)
        xb = np.ascontiguousarray(xb.transpose(3, 0, 2, 1))
        # fp8 planar k-pair moving operand: [p, c, kp, j, t]
        x8b = np.ascontiguousarray(
            xs[b].reshape(NCH, TC, KP, 2, P).transpose(4, 0, 2, 3, 1).astype(NPFP8)
        )
        in_maps.append(
            {
                "xT": xb,
                "x8": x8b,
                "wz8": wz8_t,
                "wh": wh_t,
                "wo": wo_t,
                "biases": bias_p,
            }
        )

    nc = _get_nc()
    old_m = nc.m
    nc.m = get_hw_module(nc.m)
    try:
        res = run_bass_kernel_spmd(
            nc, in_maps, core_ids=list(range(B)), trace=bool(TRACE)
        )
    finally:
        nc.m = old_m
    LAST_RESULTS = res

    out_full = np.empty((B, T, O), np.float32)
    for b in range(B):
        # [P, MO, T] bf16 -> [O, T] -> [T, O] f32
        ob = np.asarray(res.results[b]["out"], dtype=np.float32)
        out_full[b] = ob.transpose(1, 0, 2).reshape(O, T).T
    return out_full
